# revision 18
# baseline (speedup 1.0000x reference)
"""Trainium2 Bass kernel for a dense transformer block (B=2, T=2048, D=2048,
N=16 q heads, K=8 kv heads, H=128, F=8192, causal attention, RoPE, RMSNorm,
GeGLU FFN), sharded over 8 NeuronCores.

Sharding (Megatron-style TP + sequence-split FFN):
  - Each core owns 2 q heads + 1 kv head (column-split QKV with the pre-attn
    RMSNorm gain folded into the weights host-side).
  - All activations are kept feature-major (transposed, [feat, tok]) so that
    attention needs no P-matrix transposes: S^T = K @ Q^T comes straight from
    feature-major q/k, softmax denominators are ones-vector matmuls on the PE
    (no max subtraction needed: logits are O(5)), and P^T feeds the PV matmul
    directly.  Only V needs 32 small PE transposes.
  - enc heads are AllToAll'd so core c ends with all 16 heads' enc for its
    own 512-token slice; attn_vec with the full [2048,2048] w_av, then
    residual + RMSNorm + full-F GeGLU FFN for that slice (same FLOPs as
    tensor-parallel FFN, no second collective).  Host concatenates out^T.

All weights/activations are pre-swizzled host-side so every SBUF block loads
as ONE contiguous DMA (16-64KB per partition row).  The residual stays in
SBUF for the whole FFN; FFN weight streams are double-buffered and their
first block is issued before the collective so the AllToAll overlaps
independent DMA traffic.

HW scheduling rules baked in (measured on trn2, not from the cost model):
  - Engine reads of PSUM while the PE is streaming into PSUM are ~2-4x
    their SBUF cost; every matmul result is drained by exactly ONE psum
    read (Act gelu / Act or DVE copy), then all elementwise work runs
    from SBUF in bf16 (2x packed DVE mode, base-partition-matched
    operands via partition-duplicated rope tables).
  - The proj+attention phase is merged: the Act engine runs only the
    softmax exp stream (its true bottleneck); den/PV matmuls are
    deferred one iteration so the PE never waits on exp; proj chunk
    matmuls are chopped into ~8-matmul closures and interleaved between
    S matmuls as PE filler.  PSUM budget: pr(2) s(2) o(2) vt(1) den(1).
  - Two HWDGE queues (SP + Act) split by role: SP carries the head
    (xh/qkv/tables) and big weight streams, Act carries latency-critical
    post-collective loads and tail stores.
"""
import numpy as np

import concourse.bass as bass
import concourse.bacc as bacc
import concourse.tile as tile
from concourse import mybir
from concourse.bass_utils import run_bass_kernel_spmd
from concourse.masks import make_identity, make_upper_triangular

F32 = mybir.dt.float32
BF16 = mybir.dt.bfloat16
NP_BF16 = mybir.dt.np(BF16)

B, T, D = 2, 2048, 2048
N, K, H = 16, 8, 128
F = 8192
NCORES = 8
BT = B * T                  # 4096 flattened tokens (tok = b*T + t)
TOKC = BT // NCORES         # 512 tokens per chunk / per-core slice
NCHUNK = NCORES             # 8 token chunks
DT_TILES = D // 128         # 16
F_TILES = F // 128          # 64
FS_BLK = F // 256           # 32 gating column blocks
FG_BLK = F_TILES // 4       # 16 wlin row groups
GH = N // K                 # 2 q heads per kv head (= per core)
QB = T // TOKC              # 4 query chunks per batch
KVB = T // 128              # 16 kv blocks per batch
EPS = 1e-6


def build_program(n_cores=NCORES, sim=False, reps=1, no_rs=False,
                  phases=None, rep_phases=None):
    nc = bacc.Bacc("TRN2", target_bir_lowering=False, debug=False,
                   num_devices=n_cores)

    # ---- I/O (pre-swizzled host-side; see make_host_inputs) ----
    xh = nc.dram_tensor("xh", [NCHUNK, 128, DT_TILES, TOKC], BF16,
                        kind="ExternalInput")
    xTsh = nc.dram_tensor("xTsh", [128, DT_TILES, TOKC], F32,
                          kind="ExternalInput")
    wqkvh = nc.dram_tensor("wqkvh", [128, DT_TILES, 4 * H], BF16,
                           kind="ExternalInput")
    wavh = nc.dram_tensor("wavh", [128, DT_TILES, D], BF16,
                          kind="ExternalInput")
    wg0h = nc.dram_tensor("wg0h", [FS_BLK, 128, DT_TILES, 256], BF16,
                          kind="ExternalInput")
    wg1h = nc.dram_tensor("wg1h", [FS_BLK, 128, DT_TILES, 256], BF16,
                          kind="ExternalInput")
    wlinh = nc.dram_tensor("wlinh", [2, FG_BLK, 128, 4, 1024], BF16,
                           kind="ExternalInput")
    costab = nc.dram_tensor("costab", [128, BT], BF16, kind="ExternalInput")
    sintab = nc.dram_tensor("sintab", [128, BT], BF16, kind="ExternalInput")
    outT = nc.dram_tensor("outT", [D, TOKC], F32, kind="ExternalOutput")

    with tile.TileContext(nc) as tc:
        for r in range(reps):
            ph = rep_phases[r] if rep_phases is not None else phases
            _build(tc, (1 if no_rs else n_cores), sim, xh, xTsh, wqkvh,
                   wavh, wg0h, wg1h, wlinh, costab, sintab, outT,
                   phases=ph)
    nc.compile()
    return nc


def _build(tc, n_cores, sim, xh, xTsh, wqkvh, wavh, wg0h, wg1h, wlinh,
           costab, sintab, outT, phases=None):
    nc = tc.nc
    AF = mybir.ActivationFunctionType
    if phases is None:
        phases = {"proj", "attn", "av", "ffn"}

    with tc.tile_pool(name="const", bufs=1) as const, \
         tc.tile_pool(name="dram", bufs=1, space="DRAM") as dram:
        ones128 = const.tile([128, 1], BF16, tag="ones128", name="ones128")
        nc.vector.memset(ones128[:], 1.0)
        ones_row = const.tile([1, 128], F32, tag="ones_row", name="ones_row")
        nc.vector.memset(ones_row[:], 1.0)
        ident = const.tile([128, 128], BF16, tag="ident", name="ident")
        make_identity(nc, ident[:])
        # keep-mask for diagonal attention blocks on S^T [kv, q]:
        # U[p, f] = 1 if p <= f else 0
        umask = const.tile([128, 128], BF16, tag="umask", name="umask")
        make_upper_triangular(nc, umask[:], val=1.0, diag=True)
        eps1 = const.tile([1, 1], F32, tag="eps1", name="eps1")
        nc.vector.memset(eps1[:], EPS)

        # DRAM buffers for the collective
        cc_in = dram.tile([NCHUNK, 2 * H, TOKC], BF16,
                          tag="cc_in", name="cc_in")
        cc_out = dram.tile([NCHUNK, 2 * H, TOKC], BF16,
                           tag="cc_out", name="cc_out")

        with tc.tile_pool(name="persAct", bufs=1) as pers:
            if phases & {"proj", "attn", "av"}:
                qT = [pers.tile([128, BT], BF16, tag=f"qT{h}", name=f"qT{h}")
                      for h in range(GH)]
                kT = pers.tile([128, BT], BF16, tag="kT", name="kT")
                vtok = [pers.tile([128, H], BF16, tag=f"vtok{g}",
                                  name=f"vtok{g}") for g in range(2 * KVB)]
                encT = [pers.tile([128, BT], BF16, tag=f"encT{h}",
                                  name=f"encT{h}") for h in range(GH)]

            if {"proj", "attn"} <= phases:
                _phase_projattn(tc, nc, AF, xh, wqkvh, costab, sintab,
                                ones128, ones_row, ident, eps1, umask,
                                qT, kT, vtok, encT, cc_in)
            elif "proj" in phases:
                _phase_proj(tc, nc, AF, xh, wqkvh, costab, sintab,
                            ones128, ones_row, ident, eps1, qT, kT, vtok)
            elif "attn" in phases:
                _phase_attn(tc, nc, AF, ones128, ones_row, umask, qT, kT,
                            vtok, encT, cc_in)
            if "av" in phases:
                if n_cores > 1:
                    nc.gpsimd.collective_compute(
                        "AllToAll",
                        mybir.AluOpType.bypass,
                        replica_groups=[list(range(n_cores))],
                        ins=[cc_in.opt()],
                        outs=[cc_out.opt()],
                    )
                else:
                    nc.sync.dma_start(cc_out[:], cc_in[:])

        if "ffn" in phases:
            _phase_ffn(tc, nc, AF, sim, xTsh, wavh, wg0h, wg1h, wlinh,
                       cc_out, ones128, ones_row, eps1, outT)


def _phase_projattn(tc, nc, AF, xh, wqkvh, costab, sintab, ones128,
                    ones_row, ident, eps1, umask, qT, kT, vtok, encT,
                    cc_in):
    """Merged projection+attention with the Act engine dedicated to the
    exp stream (the phase's true bottleneck, ~1.2us per [128,512] psum
    read on HW).

    Structure: 16 attention iterations (b, h, c) in order, with den/PV
    matmuls DEFERRED by one iteration (their p = exp(S) inputs were
    produced last iteration, so the PE never waits on the Act engine).
    Projection chunk k's matmul groups are chopped into ~8-matmul
    closures and interleaved between the S matmuls, so the PE stream
    always has ready work while exp drains, and S tiles are produced
    just-in-time for the exp stream (s_ps rotates through 2 banks).

    PSUM budget (8 banks): proj pr0/pr1 (2) + s_ps (2) + o_ps (2) +
    den_ps (2).  V transposes use the DMA xbar instead of the PE/PSUM.
    All proj psum drains run on DVE (bf16 SBUF rope at 2x packed rate);
    Act runs ONLY exps."""
    iters = [(b, h, c) for b in range(B) for c in range(QB)
             for h in range(GH)]

    with tc.tile_pool(name="ptab", bufs=1) as ptab, \
         tc.tile_pool(name="pAB", bufs=2) as pAB, \
         tc.tile_pool(name="pSB", bufs=3) as pSB, \
         tc.tile_pool(name="pP", bufs=36) as pP, \
         tc.tile_pool(name="pC2", bufs=2) as pC2, \
         tc.tile_pool(name="psPR", bufs=1, space="PSUM") as psPR, \
         tc.tile_pool(name="psS", bufs=2, space="PSUM") as psS, \
         tc.tile_pool(name="psC", bufs=2, space="PSUM") as psC, \
         tc.tile_pool(name="psVT", bufs=1, space="PSUM") as psVT, \
         tc.tile_pool(name="psC1", bufs=1, space="PSUM") as psC1:
        wqkv_t = ptab.tile([128, DT_TILES, 4 * H], BF16, tag="wqkv",
                           name="wqkv")
        nc.sync.dma_start(wqkv_t[:], wqkvh[:])
        cos_t = ptab.tile([128, BT], BF16, tag="cos_t", name="cos_t")
        nc.sync.dma_start(cos_t[:], costab[:])
        sin_t = ptab.tile([128, BT], BF16, tag="sin_t", name="sin_t")
        nc.sync.dma_start(sin_t[:], sintab[:])

        def emit_proj_chunk(c):
            """Return a list of closures (PE mm batches + elementwise
            drains) for projection chunk c."""
            cols = slice(c * TOKC, (c + 1) * TOKC)
            xc = pAB.tile([128, DT_TILES, TOKC], BF16, tag="xc", name="xc")
            (nc.sync if c % 2 == 0 else nc.scalar).dma_start(xc[:], xh[c])
            closures = []
            for fp in range(2):
                pr = [psPR.tile([128, TOKC], F32, tag=f"pr{i}",
                                name=f"pr{i}") for i in range(2)]

                def mm_batch(pr=pr, fp=fp, kts=None, xc=xc):
                    for kt in kts:
                        for i in range(2):
                            ft = 2 * fp + i
                            nc.tensor.matmul(
                                pr[i][:],
                                wqkv_t[:, kt, ft * 128:(ft + 1) * 128],
                                xc[:, kt, :],
                                start=(kt == 0),
                                stop=(kt == DT_TILES - 1))
                for k0 in range(0, DT_TILES, 4):
                    closures.append(
                        lambda f=mm_batch, ks=range(k0, k0 + 4): f(kts=ks))

                def drain(pr=pr, fp=fp, cols=cols, c=c):
                    for i in range(2):
                        ft = 2 * fp + i
                        ps = pr[i]
                        sb = pSB.tile([128, TOKC], BF16, tag="sb",
                                      name="sb")
                        nc.vector.tensor_copy(sb[:], ps[:])
                        if ft < 3:
                            dst = qT[ft] if ft < GH else kT
                            t1 = pAB.tile([64, TOKC], BF16, tag="t1",
                                          name="t1")
                            t2 = pAB.tile([64, TOKC], BF16, tag="t2",
                                          name="t2")
                            nc.vector.tensor_mul(t1[:], sb[0:64, :],
                                                 cos_t[0:64, cols])
                            nc.vector.tensor_mul(t2[:], sb[64:128, :],
                                                 sin_t[64:128, cols])
                            nc.vector.tensor_sub(dst[0:64, cols],
                                                 t1[:], t2[:])
                            t3 = pAB.tile([64, TOKC], BF16, tag="t3",
                                          name="t3")
                            t4 = pAB.tile([64, TOKC], BF16, tag="t4",
                                          name="t4")
                            nc.vector.tensor_mul(t3[:], sb[64:128, :],
                                                 cos_t[64:128, cols])
                            nc.vector.tensor_mul(t4[:], sb[0:64, :],
                                                 sin_t[0:64, cols])
                            nc.vector.tensor_add(dst[64:128, cols],
                                                 t3[:], t4[:])
                        else:
                            # v: token-major via PE transpose (identity)
                            for j in range(TOKC // 128):
                                vt_ps = psVT.tile([128, 128], BF16,
                                                  tag="vt_ps",
                                                  name="vt_ps")
                                nc.tensor.transpose(
                                    vt_ps[:],
                                    sb[:, j * 128:(j + 1) * 128],
                                    ident[:])
                                nc.vector.tensor_copy(vtok[c * 4 + j][:],
                                                      vt_ps[:])
                closures.append(drain)
            return closures

        def livesl(b, c, j, cq):
            d = j - 4 * c
            lo = d * 128 if d > 0 else 0
            return (slice(lo, TOKC),
                    slice(cq * TOKC + lo, (cq + 1) * TOKC))

        pending = None
        for step in range(len(iters) + 2):
            filler = []
            if pending is not None:
                (pb, ph, pc, pcq, pnkv, p_list, o_ps, den_ps) = pending

                def denpv(j, pb=pb, pc=pc, pcq=pcq, pnkv=pnkv,
                          p_list=p_list, o_ps=o_ps, den_ps=den_ps):
                    live, _ = livesl(pb, pc, j, pcq)
                    g = pb * KVB + j
                    p_sb = p_list[j]
                    nc.tensor.matmul(den_ps[:, live], ones128[:],
                                     p_sb[:, live],
                                     start=(j == 0), stop=(j == pnkv - 1))
                    nc.tensor.matmul(o_ps[:, live], vtok[g][:],
                                     p_sb[:, live],
                                     start=(j == 0), stop=(j == pnkv - 1))
                for j in range(pnkv):
                    filler.append(lambda f=denpv, j=j: f(j))
            if step < NCHUNK:
                filler.extend(emit_proj_chunk(step))

            # attention iteration lags proj-chunk emission by one step so
            # every S matmul reads qT/kT columns already written
            cur = iters[step - 1] if 1 <= step <= len(iters) else None
            if cur is not None:
                b, h, c = cur
                cq = b * QB + c
                nkv = 4 * c + 4
                o_ps = psC.tile([128, TOKC], F32, tag="o_ps",
                                name=f"o_ps{step}")
                den_ps = psC1.tile([1, TOKC], F32, tag="den_ps",
                                   name=f"den_ps{step}")
                p_list = []
                nf = len(filler)
                fi = 0
                for j in range(nkv):
                    live, qlive = livesl(b, c, j, cq)
                    g = b * KVB + j
                    d = j - 4 * c
                    s_ps = psS.tile([128, TOKC], F32, tag="s_ps",
                                    name="s_ps")
                    nc.tensor.matmul(
                        s_ps[:, live], kT[:, g * 128:(g + 1) * 128],
                        qT[h][:, qlive], start=True, stop=True)
                    p_sb = pP.tile([128, TOKC], BF16, tag="p_sb",
                                   name="p_sb")
                    nc.scalar.activation(p_sb[:, live], s_ps[:, live],
                                         AF.Exp)
                    if d >= 0:
                        nc.vector.tensor_mul(
                            p_sb[:, d * 128:(d + 1) * 128],
                            p_sb[:, d * 128:(d + 1) * 128], umask[:])
                    p_list.append(p_sb)
                    # spread filler work between S matmuls (front-loaded)
                    want = (nf * (j + 1) + nkv - 1) // nkv
                    while fi < want:
                        filler[fi]()
                        fi += 1
                while fi < nf:
                    filler[fi]()
                    fi += 1
            else:
                for f in filler:
                    f()

            # finish the pending iteration: normalize + stage to cc_in
            if pending is not None:
                (qb, qh, qc, qcq, qnkv, q_list, q_o, q_den) = pending
                qcols = slice(qcq * TOKC, (qcq + 1) * TOKC)
                rec = pC2.tile([1, TOKC], F32, tag="rec", name="rec")
                nc.vector.reciprocal(rec[:], q_den[:])
                db = pC2.tile([128, TOKC], F32, tag="db", name="db")
                nc.gpsimd.partition_broadcast(db[:], rec[:], channels=128)
                nc.vector.tensor_mul(encT[qh][:, qcols], q_o[:], db[:])
                nc.sync.dma_start(cc_in[qcq, qh * H:(qh + 1) * H, :],
                                  encT[qh][:, qcols])
            if cur is not None:
                pending = (b, h, c, cq, nkv, p_list, o_ps, den_ps)
            else:
                pending = None


def _phase_proj(tc, nc, AF, xh, wqkvh, costab, sintab,
                ones128, ones_row, ident, eps1, qT, kT, vtok):
    """QKV projection + RoPE, feature-major.  The pre-attention RMSNorm is
    folded host-side: xh is already x * rsqrt(mean(x^2)+eps) (the (1+scale)
    gain lives in the weights), so no on-device norm chain is needed.

    PSUM discipline: each qkv psum tile is read EXACTLY ONCE, by the
    Activation engine (psum->sbuf bf16 copy); all RoPE elementwise work runs
    on DVE from SBUF in bf16 (2x packed mode).  DVE reads of PSUM while the
    PE streams into PSUM serialize badly on HW (measured 3x phase cost)."""
    with tc.tile_pool(name="ptab", bufs=1) as ptab, \
         tc.tile_pool(name="pAB", bufs=2) as pAB, \
         tc.tile_pool(name="pSB", bufs=3) as pSB, \
         tc.tile_pool(name="psVT", bufs=2, space="PSUM") as psVT, \
         tc.tile_pool(name="psPR", bufs=3, space="PSUM") as psPR:
        # resident qkv weights, one DMA
        wqkv_t = ptab.tile([128, DT_TILES, 4 * H], BF16, tag="wqkv",
                           name="wqkv")
        nc.sync.dma_start(wqkv_t[:], wqkvh[:])
        # resident rope tables (bf16, duplicated to both partition halves
        # so every DVE product reads base-matched SBUF operands)
        cos_t = ptab.tile([128, BT], BF16, tag="cos_t", name="cos_t")
        nc.sync.dma_start(cos_t[:], costab[:])
        sin_t = ptab.tile([128, BT], BF16, tag="sin_t", name="sin_t")
        nc.sync.dma_start(sin_t[:], sintab[:])

        for c in range(NCHUNK):
            cols = slice(c * TOKC, (c + 1) * TOKC)
            xc = pAB.tile([128, DT_TILES, TOKC], BF16, tag="xc", name="xc")
            nc.sync.dma_start(xc[:], xh[c])

            # qkv projection in two ft-pair passes (psum: 2 tags x bufs=3)
            for fp in range(2):
                pr = [psPR.tile([128, TOKC], F32, tag=f"pr{i}",
                                name=f"pr{i}") for i in range(2)]
                for kt in range(DT_TILES):
                    for i in range(2):
                        ft = 2 * fp + i
                        nc.tensor.matmul(
                            pr[i][:],
                            wqkv_t[:, kt, ft * 128:(ft + 1) * 128],
                            xc[:, kt, :],
                            start=(kt == 0), stop=(kt == DT_TILES - 1))
                for i in range(2):
                    ft = 2 * fp + i
                    ps = pr[i]
                    if ft < 3:
                        # single psum read: Act copies to SBUF bf16
                        sb = pSB.tile([128, TOKC], BF16, tag="sb",
                                      name="sb")
                        nc.scalar.copy(sb[:], ps[:])
                        # rope on DVE, all-bf16 SBUF operands
                        dst = qT[ft] if ft < GH else kT
                        t1 = pAB.tile([64, TOKC], BF16, tag="t1", name="t1")
                        t2 = pAB.tile([64, TOKC], BF16, tag="t2", name="t2")
                        nc.vector.tensor_mul(t1[:], sb[0:64, :],
                                             cos_t[0:64, cols])
                        nc.vector.tensor_mul(t2[:], sb[64:128, :],
                                             sin_t[64:128, cols])
                        nc.vector.tensor_sub(dst[0:64, cols], t1[:], t2[:])
                        t3 = pAB.tile([64, TOKC], BF16, tag="t3", name="t3")
                        t4 = pAB.tile([64, TOKC], BF16, tag="t4", name="t4")
                        nc.vector.tensor_mul(t3[:], sb[64:128, :],
                                             cos_t[64:128, cols])
                        nc.vector.tensor_mul(t4[:], sb[0:64, :],
                                             sin_t[0:64, cols])
                        nc.vector.tensor_add(dst[64:128, cols], t3[:], t4[:])
                    else:
                        # v: cast to bf16 via Act, transpose to token-major
                        vsb = pAB.tile([128, TOKC], BF16, tag="vsb",
                                       name="vsb")
                        nc.scalar.copy(vsb[:], ps[:])
                        for j in range(TOKC // 128):
                            vt_ps = psVT.tile([128, 128], BF16, tag="vt_ps",
                                              name="vt_ps")
                            nc.tensor.transpose(
                                vt_ps[:], vsb[:, j * 128:(j + 1) * 128],
                                ident[:])
                            nc.vector.tensor_copy(vtok[c * 4 + j][:],
                                                  vt_ps[:])


def _phase_attn(tc, nc, AF, ones128, ones_row, umask, qT, kT, vtok, encT,
                cc_in):
    """Causal attention in the transposed (S^T) formulation, software-
    pipelined by one kv block: the S matmul for block j+1 is issued BEFORE
    den/pv of block j, so the PE computes S(j+1) while the Act engine's
    exp(j) (the only psum reader) drains.  Each finished (head, chunk)
    slice is staged to cc_in immediately so the DMA spreads over the
    phase."""
    with tc.tile_pool(name="pC", bufs=6) as pC, \
         tc.tile_pool(name="pC2", bufs=2) as pC2, \
         tc.tile_pool(name="psS", bufs=4, space="PSUM") as psS, \
         tc.tile_pool(name="psC", bufs=2, space="PSUM") as psC, \
         tc.tile_pool(name="psC1", bufs=2, space="PSUM") as psC1:
        for b in range(B):
            for c in range(QB):
                cq = b * QB + c
                qcols = slice(cq * TOKC, (cq + 1) * TOKC)
                nkv = 4 * c + 4   # kv blocks 0 .. 4c+3

                def livesl(j):
                    # diagonal blocks (d>0): queries < d*128 are fully
                    # masked; restrict work to live columns.  j=0 is
                    # full-width so psum has_written covers every col.
                    d = j - 4 * c
                    lo = d * 128 if d > 0 else 0
                    return (slice(lo, TOKC),
                            slice(cq * TOKC + lo, (cq + 1) * TOKC))

                def s_matmul(h, j):
                    live, qlive = livesl(j)
                    g = b * KVB + j
                    s_ps = psS.tile([128, TOKC], F32, tag="s_ps",
                                    name="s_ps")
                    nc.tensor.matmul(
                        s_ps[:, live], kT[:, g * 128:(g + 1) * 128],
                        qT[h][:, qlive], start=True, stop=True)
                    return s_ps

                # two interleaved head-streams: while the Act engine's
                # exp(h=0) drains, the PE works on stream h=1 and vice
                # versa, so neither engine waits on the other's latency.
                o_ps = [psC.tile([128, TOKC], F32, tag="o_ps",
                                 name=f"o_ps{h}") for h in range(GH)]
                den_ps = [psC1.tile([1, TOKC], F32, tag="den_ps",
                                    name=f"den_ps{h}") for h in range(GH)]
                s_cur = [s_matmul(h, 0) for h in range(GH)]
                for j in range(nkv):
                    live, qlive = livesl(j)
                    g = b * KVB + j
                    d = j - 4 * c
                    for h in range(GH):
                        p_sb = pC.tile([128, TOKC], BF16, tag="p_sb",
                                       name="p_sb")
                        nc.scalar.activation(p_sb[:, live],
                                             s_cur[h][:, live], AF.Exp)
                        if d >= 0:
                            nc.vector.tensor_mul(
                                p_sb[:, d * 128:(d + 1) * 128],
                                p_sb[:, d * 128:(d + 1) * 128], umask[:])
                        if j + 1 < nkv:
                            s_cur[h] = s_matmul(h, j + 1)
                        nc.tensor.matmul(den_ps[h][:, live], ones128[:],
                                         p_sb[:, live],
                                         start=(j == 0),
                                         stop=(j == nkv - 1))
                        nc.tensor.matmul(o_ps[h][:, live], vtok[g][:],
                                         p_sb[:, live],
                                         start=(j == 0),
                                         stop=(j == nkv - 1))
                # normalize: enc = O / denom (den broadcast on Pool, no
                # PE/psum involvement)
                for h in range(GH):
                    rec = pC2.tile([1, TOKC], F32, tag="rec", name="rec")
                    nc.vector.reciprocal(rec[:], den_ps[h][:])
                    db = pC2.tile([128, TOKC], F32, tag="db", name="db")
                    nc.gpsimd.partition_broadcast(db[:], rec[:],
                                                  channels=128)
                    nc.vector.tensor_mul(encT[h][:, qcols], o_ps[h][:],
                                         db[:])
                    nc.scalar.dma_start(
                        cc_in[cq, h * H:(h + 1) * H, :], encT[h][:, qcols])


def _phase_ffn(tc, nc, AF, sim, xTsh, wavh, wg0h, wg1h, wlinh, cc_out,
               ones128, ones_row, eps1, outT):
    """residual + RMSNorm + GeGLU FFN on this core's 512-token slice.
    Residual stays in SBUF; FFN weight streams are double-buffered with the
    first gating block prefetched before the collective lands."""
    with tc.tile_pool(name="pRes", bufs=1) as pres, \
         tc.tile_pool(name="pWg", bufs=2) as pWg, \
         tc.tile_pool(name="pE", bufs=2) as pE, \
         tc.tile_pool(name="pE3", bufs=3) as pE3:
        # res_all doubles as the x^T slice buffer (residual accumulates
        # in place); hn_all holds the normalized activations in bf16.
        res_all = pres.tile([128, DT_TILES, TOKC], F32, tag="res_all",
                            name="res_all")
        nc.scalar.dma_start(res_all[:], xTsh[:])
        hn_all = pres.tile([128, DT_TILES, TOKC], BF16, tag="hn_all",
                           name="hn_all")

        with tc.tile_pool(name="pAV", bufs=1) as pAV, \
             tc.tile_pool(name="psE", bufs=1, space="PSUM") as psE, \
             tc.tile_pool(name="psAV", bufs=3, space="PSUM") as psAV:
            wavf = pAV.tile([128, DT_TILES, D], BF16, tag="wavf",
                            name="wavf")
            nc.scalar.dma_start(wavf[:], wavh[:])
            # enc for my token slice lands in hn_all's slots: encf slot kt
            # dies at the last attn_vec matmul, before hn_all[:, kt, :] is
            # written after the norm barrier (the Tile WAR dep orders them)
            for j in range(NCHUNK):
                for h in range(GH):
                    nc.scalar.dma_start(hn_all[:, 2 * j + h, :],
                                        cc_out[j, h * H:(h + 1) * H, :])
            sq_all = pAV.tile([128, DT_TILES, TOKC], BF16, tag="sq_all",
                              name="sq_all")
            for dt in range(DT_TILES):
                ao_ps = psAV.tile([128, TOKC], F32, tag="ao_ps", name="ao_ps")
                for kt in range(DT_TILES):
                    nc.tensor.matmul(ao_ps[:],
                                     wavf[:, kt, dt * 128:(dt + 1) * 128],
                                     hn_all[:, kt, :],
                                     start=(kt == 0),
                                     stop=(kt == DT_TILES - 1))
                # single psum read via Act; DVE works from SBUF
                ao_sb = pE3.tile([128, TOKC], BF16, tag="ao_sb",
                                 name="ao_sb")
                nc.scalar.copy(ao_sb[:], ao_ps[:])
                nc.vector.tensor_add(res_all[:, dt, :], ao_sb[:],
                                     res_all[:, dt, :])
                nc.vector.tensor_mul(sq_all[:, dt, :], res_all[:, dt, :],
                                     res_all[:, dt, :])
            # batched so the in-order PE queue is never blocked mid-attn_vec
            ssq2 = psE.tile([1, TOKC], F32, tag="ssq2", name="ssq2")
            for dt in range(DT_TILES):
                nc.tensor.matmul(ssq2[:], ones128[:], sq_all[:, dt, :],
                                 start=(dt == 0), stop=(dt == DT_TILES - 1))
            sd2 = pE.tile([1, TOKC], F32, tag="sd2", name="sd2")
            nc.scalar.activation(sd2[:], ssq2[:], AF.Sqrt,
                                 bias=eps1[:], scale=1.0 / D)
            rr2 = pE.tile([1, TOKC], F32, tag="rr2", name="rr2")
            nc.vector.reciprocal(rr2[:], sd2[:])
            # broadcast on Pool: no PE involvement, PE stays on matmuls
            r2b = pE.tile([128, TOKC], F32, tag="r2b", name="r2b")
            nc.gpsimd.partition_broadcast(r2b[:], rr2[:], channels=128)

        # gate path: act = gelu_tanh(hn @ wg0) * (hn @ wg1), act in SBUF
        with tc.tile_pool(name="pActs", bufs=1) as pActs:
          act = pActs.tile([128, F_TILES, TOKC], BF16, tag="act",
                           name="act")
          with tc.tile_pool(name="pWg", bufs=2) as pWg, \
               tc.tile_pool(name="pTmp", bufs=2) as pTmp, \
               tc.tile_pool(name="psW", bufs=3, space="PSUM") as psW:

            def load_wg(fs):
                w0 = pWg.tile([128, DT_TILES, 256], BF16, tag="w0s",
                              name=f"w0s{fs}")
                nc.sync.dma_start(w0[:], wg0h[fs])
                w1 = pWg.tile([128, DT_TILES, 256], BF16, tag="w1s",
                              name=f"w1s{fs}")
                nc.sync.dma_start(w1[:], wg1h[fs])
                return w0, w1

            cur = load_wg(0)   # overlaps the hn muls below
            # split the norm muls across DVE and Pool to halve the serial
            # tail the PE waits on before the gating stream starts
            for dt in range(DT_TILES):
                eng = nc.vector if dt % 2 == 0 else nc.gpsimd
                eng.tensor_mul(hn_all[:, dt, :], res_all[:, dt, :],
                               r2b[:])
            for fs in range(FS_BLK):
                w0s, w1s = cur
                if fs + 1 < FS_BLK:
                    cur = load_wg(fs + 1)
                for fi in range(2):
                    f = fs * 2 + fi
                    g_ps = psW.tile([128, TOKC], F32, tag="g_ps", name="g_ps")
                    u_ps = psW.tile([128, TOKC], F32, tag="u_ps", name="u_ps")
                    for kt in range(DT_TILES):
                        nc.tensor.matmul(g_ps[:],
                                         w0s[:, kt, fi * 128:(fi + 1) * 128],
                                         hn_all[:, kt, :],
                                         start=(kt == 0),
                                         stop=(kt == DT_TILES - 1))
                    for kt in range(DT_TILES):
                        nc.tensor.matmul(u_ps[:],
                                         w1s[:, kt, fi * 128:(fi + 1) * 128],
                                         hn_all[:, kt, :],
                                         start=(kt == 0),
                                         stop=(kt == DT_TILES - 1))
                    gg = pE3.tile([128, TOKC], BF16, tag="gg", name="gg")
                    if not sim:
                        nc.scalar.activation(gg[:], g_ps[:],
                                             AF.Gelu_apprx_tanh)
                    else:
                        # tanh-gelu composite (CoreSim has no Gelu LUT)
                        ga = pTmp.tile([128, TOKC], F32, tag="ga", name="ga")
                        nc.vector.tensor_mul(ga[:], g_ps[:], g_ps[:])
                        nc.vector.tensor_mul(ga[:], ga[:], g_ps[:])
                        nc.vector.tensor_scalar(ga[:], ga[:], 0.044715,
                                                None, mybir.AluOpType.mult)
                        nc.vector.tensor_add(ga[:], ga[:], g_ps[:])
                        gb = pTmp.tile([128, TOKC], F32, tag="gb", name="gb")
                        nc.scalar.activation(gb[:], ga[:], AF.Tanh,
                                             scale=0.7978845608028654)
                        nc.vector.tensor_scalar(gb[:], gb[:], 1.0, 0.5,
                                                mybir.AluOpType.add,
                                                mybir.AluOpType.mult)
                        nc.vector.tensor_mul(gg[:], gb[:], g_ps[:])
                    nc.vector.tensor_mul(act[:, f, :], u_ps[:], gg[:])

          # linear: out^T[dt] = sum_f wlin[f, dt-cols].T @ act[f] + residual
          with tc.tile_pool(name="pL", bufs=4) as pL, \
               tc.tile_pool(name="psL", bufs=1, space="PSUM") as psL:
              for pas in range(2):       # dt 0-7, then 8-15
                  o_ps = [psL.tile([128, TOKC], F32, tag=f"o_ps{i}",
                                   name=f"o_ps{i}") for i in range(8)]
                  for fg in range(FG_BLK):
                      wl = pL.tile([128, 4, 1024], BF16, tag="wl",
                                   name=f"wl{pas}_{fg}")
                      (nc.sync if pas == 0 else nc.scalar).dma_start(
                          wl[:], wlinh[pas, fg])
                      for fj in range(4):
                          f = fg * 4 + fj
                          for i in range(8):
                              nc.tensor.matmul(
                                  o_ps[i][:],
                                  wl[:, fj, i * 128:(i + 1) * 128],
                                  act[:, f, :],
                                  start=(f == 0), stop=(f == F_TILES - 1))
                  for i in range(8):
                      dt = pas * 8 + i
                      ob = pE3.tile([128, TOKC], F32, tag="ob", name="ob")
                      nc.vector.tensor_add(ob[:], o_ps[i][:],
                                           res_all[:, dt, :])
                      nc.scalar.dma_start(
                          outT[dt * 128:(dt + 1) * 128, :], ob[:])


# ---------------------------------------------------------------------------
# Host side
# ---------------------------------------------------------------------------
def make_host_inputs(x, positions, w_q, w_kv, w_attn_vec, scale_pre_attn,
                     scale_pre_ffw, w_gating, w_linear):
    """Build the per-core input maps (all numpy, pre-swizzled layouts)."""
    x = np.asarray(x, np.float32)
    positions = np.asarray(positions)
    w_q = np.asarray(w_q, np.float32)
    w_kv = np.asarray(w_kv, np.float32)
    w_attn_vec = np.asarray(w_attn_vec, np.float32)
    s1 = 1.0 + np.asarray(scale_pre_attn, np.float32)
    s2 = 1.0 + np.asarray(scale_pre_ffw, np.float32)
    w_gating = np.asarray(w_gating, np.float32)
    w_linear = np.asarray(w_linear, np.float32)

    xT = np.ascontiguousarray(x.reshape(BT, D).T)          # [D, BT] f32
    # pre-attn RMSNorm folded host-side: xn = x * rsqrt(mean(x^2) + eps)
    rr = 1.0 / np.sqrt(np.mean(np.square(x), axis=-1) + EPS)   # [B, T]
    xnT = xT * rr.reshape(BT)[None, :]
    # xh[c, p, kt, n] = xnT[kt*128+p, c*TOKC+n]
    xh = np.ascontiguousarray(
        xnT.reshape(DT_TILES, 128, NCHUNK, TOKC).transpose(2, 1, 0, 3)
    ).astype(NP_BF16)

    pos = positions.reshape(BT).astype(np.float32)         # [BT]
    half = H // 2
    timescale = (10000.0 ** ((2.0 / H) * np.arange(half, dtype=np.float32)))
    rad = pos[None, :] / timescale[:, None]                # [64, BT]
    costab = np.concatenate([np.cos(rad), np.cos(rad)], 0).astype(NP_BF16)
    sintab = np.concatenate([np.sin(rad), np.sin(rad)], 0).astype(NP_BF16)

    wg0 = (w_gating[0] * s2[:, None]).astype(NP_BF16)      # [D, F]
    wg1 = (w_gating[1] * s2[:, None]).astype(NP_BF16)
    # wg{i}h[fs, p, kt, c] = wg{i}[kt*128+p, fs*512+c]
    wg0h = np.ascontiguousarray(
        wg0.reshape(DT_TILES, 128, FS_BLK, 256).transpose(2, 1, 0, 3))
    wg1h = np.ascontiguousarray(
        wg1.reshape(DT_TILES, 128, FS_BLK, 256).transpose(2, 1, 0, 3))
    # wlinh[pas, fg, p, fj, c] = w_linear[fg*512+fj*128+p, pas*1024+c]
    wlinh = np.ascontiguousarray(
        w_linear.astype(NP_BF16).reshape(FG_BLK, 4, 128, 2, 1024)
        .transpose(3, 0, 2, 1, 4))
    # wavh[p, kt, c] = w_attn_vec.reshape(N*H, D)[kt*128+p, c]
    wavh = np.ascontiguousarray(
        w_attn_vec.reshape(N * H, D).astype(NP_BF16)
        .reshape(DT_TILES, 128, D).transpose(1, 0, 2))

    in_maps = []
    for c in range(NCORES):
        hq0, hq1 = 2 * c, 2 * c + 1
        wq0 = w_q[hq0] * s1[:, None] * (H ** -0.5)
        wq1 = w_q[hq1] * s1[:, None] * (H ** -0.5)
        wk = w_kv[0, c] * s1[:, None]
        wv = w_kv[1, c] * s1[:, None]
        wqkv_c = np.concatenate([wq0, wq1, wk, wv], axis=1).astype(NP_BF16)
        # wqkvh[p, kt, c] = wqkv_c[kt*128+p, c]
        wqkvh = np.ascontiguousarray(
            wqkv_c.reshape(DT_TILES, 128, 4 * H).transpose(1, 0, 2))
        # xTsh[p, dt, n] = xT[dt*128+p, core_slice n] (f32)
        xTs = xT[:, c * TOKC:(c + 1) * TOKC]
        xTsh = np.ascontiguousarray(
            xTs.reshape(DT_TILES, 128, TOKC).transpose(1, 0, 2))
        in_maps.append({
            "xh": xh,
            "xTsh": xTsh,
            "wqkvh": wqkvh,
            "wavh": wavh,
            "wg0h": wg0h,
            "wg1h": wg1h,
            "wlinh": wlinh,
            "costab": costab,
            "sintab": sintab,
        })
    return in_maps


def assemble_output(results):
    """results: list of per-core {"outT": [D, TOKC] f32} -> [B, T, D] f32."""
    outT = np.concatenate([np.asarray(r["outT"]) for r in results], axis=1)
    return np.ascontiguousarray(outT.T.reshape(B, T, D)).astype(np.float32)


_CACHE = {}


def _get_program():
    if "nc" not in _CACHE:
        _CACHE["nc"] = build_program(NCORES)
    return _CACHE["nc"]


def kernel(x, positions, attn_mask, w_q, w_kv, w_attn_vec, scale_pre_attn,
           scale_pre_ffw, w_gating, w_linear):
    nc = _get_program()
    in_maps = make_host_inputs(x, positions, w_q, w_kv, w_attn_vec,
                               scale_pre_attn, scale_pre_ffw, w_gating,
                               w_linear)
    _CACHE["in_maps"] = in_maps
    res = run_bass_kernel_spmd(nc, in_maps, list(range(NCORES)))
    return assemble_output(res.results)

